# revision 1
# baseline (speedup 1.0000x reference)
"""Bass/Trainium2 kernel for nn_BiChannelAttention (single-query local-window attention).

Math (per batch b, head h, with S=2049, window W=256, cutoff=S-W=1793):
  Positions before the cutoff receive a -1e6 additive mask, so after softmax their
  weight is exactly 0.0 in fp32 (exp underflows). Only the last W positions matter.

  For window rows X [W, 128] (last 255 cache rows + content row):
    q   = cnt_h @ Wq_h                      (128)
    kq  = (Wk_h/sqrt(128))^T q              (128)      <- folds Wk into q
    sc  = X kq  (+ per-position bias)       (W)        <- column-major on chip
    a   = exp(sc)          (no max-subtraction needed: unmasked scores are O(1))
    xa  = X^T a / sum(a)                    (128)
    out = Wv_h^T xa + cnt_h                 (128)

Sharding: tensor-parallel over heads, 2 heads per core x 8 cores. Each core reads
only its heads' weight slices and window slices (~2.2 MB).
"""

import sys
import numpy as np

for _p in ("/opt/trn_rl_repo", "/root/.axon_site/_ro/trn_rl_repo"):
    if _p not in sys.path:
        sys.path.insert(0, _p)

import concourse.bass as bass
import concourse.bacc as bacc
import concourse.mybir as mybir
from concourse.tile import TileContext
from concourse.bass_utils import run_bass_kernel_spmd

F32 = mybir.dt.float32
P = 128          # partitions / head_dim
B = 8            # batch
H = 16           # heads total
HPC = 2          # heads per core
NCORES = 8
T = 2048
S = T + 1
W = 256          # local attention window
CUTOFF = S - W   # 1793
NEG = -1000000.0

_NC_CACHE = {}


def _build_nc():
    nc = bacc.Bacc(None, target_bir_lowering=False, debug=False)
    # packed constants along the free dim: ident | ones | bias | cnt | (wq,wkt,wv) x HPC
    CK = 2 * P + 2 * B + HPC * B + 3 * HPC * P
    x_in = nc.declare_dram_parameter("x", [B, HPC, W, P], F32, isOutput=False)
    consts_in = nc.declare_dram_parameter("consts", [P, CK], F32, isOutput=False)
    out_t = nc.declare_dram_parameter("out", [HPC, P, B], F32, isOutput=True)

    with TileContext(nc) as tc:
        with (
            tc.tile_pool(name="const", bufs=1) as cpool,
            tc.tile_pool(name="xin", bufs=10) as xpool,
            tc.tile_pool(name="xt", bufs=10) as xtpool,
            tc.tile_pool(name="small", bufs=2) as spool,
            tc.tile_pool(name="ps_t", bufs=2, space="PSUM") as pst,
            tc.tile_pool(name="ps_qk", bufs=2, space="PSUM") as psqk,
            tc.tile_pool(name="ps_at", bufs=2, space="PSUM") as psat,
            tc.tile_pool(name="ps_xo", bufs=2, space="PSUM") as psxo,
        ):
            consts = cpool.tile([P, CK], F32, tag="consts")
            nc.sync.dma_start(out=consts[:, :], in_=consts_in[:, :])
            o = 0
            ident = consts[:, o:o + P]; o += P
            ones = consts[:, o:o + P]; o += P
            biasT = consts[:, o:o + 2 * B]; o += 2 * B
            cntT = consts[:, o:o + HPC * B]; o += HPC * B
            wq, wkt, wv = [], [], []
            for j in range(HPC):
                wq.append(consts[:, o:o + P]); o += P
                wkt.append(consts[:, o:o + P]); o += P
                wv.append(consts[:, o:o + P]); o += P

            for j in range(HPC):
                cnt_j = cntT[:, j * B:(j + 1) * B]

                # q for all 8 batches: q[e,b] = sum_d Wq[d,e] cnt[d,b]
                qk_ps = psqk.tile([P, 2 * B], F32, tag="qk")
                nc.tensor.matmul(qk_ps[:, 0:B], wq[j], cnt_j, start=True, stop=True)
                q_sb = spool.tile([P, B], F32, tag="q")
                nc.vector.tensor_copy(q_sb[:, :], qk_ps[:, 0:B])
                # kq[d,b] = sum_e WkT[e,d] q[e,b]   (WkT pre-scaled by 1/sqrt(128))
                nc.tensor.matmul(qk_ps[:, B:2 * B], wkt[j], q_sb[:, :], start=True, stop=True)
                kq_sb = spool.tile([P, B], F32, tag="kq")
                nc.vector.tensor_copy(kq_sb[:, :], qk_ps[:, B:2 * B])

                at_ps = psat.tile([P, 3 * B], F32, tag="at")  # scores [0:16], denom [16:24]
                xo_ps = psxo.tile([P, 2 * B], F32, tag="xo")  # xa [0:8], out [8:16]

                x0s, x1s, xt0s, xt1s = [], [], [], []
                for b in range(B):
                    x0 = xpool.tile([P, P], F32, tag="x0")
                    nc.sync.dma_start(out=x0[:, :], in_=x_in[b, j, 0:P, :])
                    x1 = xpool.tile([P, P], F32, tag="x1")
                    nc.sync.dma_start(out=x1[:, :], in_=x_in[b, j, P:W, :])
                    xt_ps = pst.tile([P, 2 * P], F32, tag="xtp")
                    nc.tensor.transpose(xt_ps[:, 0:P], x0[:, :], ident)
                    nc.tensor.transpose(xt_ps[:, P:2 * P], x1[:, :], ident)
                    xt0 = xtpool.tile([P, P], F32, tag="xt0")
                    nc.vector.tensor_copy(xt0[:, :], xt_ps[:, 0:P])
                    xt1 = xtpool.tile([P, P], F32, tag="xt1")
                    nc.scalar.copy(xt1[:, :], xt_ps[:, P:2 * P])
                    # scores: column [s,1] per (tile, b) -> at_ps col jt*8+b
                    nc.tensor.matmul(at_ps[:, b:b + 1], xt0[:, :], kq_sb[:, b:b + 1], start=True, stop=True)
                    nc.tensor.matmul(at_ps[:, B + b:B + b + 1], xt1[:, :], kq_sb[:, b:b + 1], start=True, stop=True)
                    x0s.append(x0); x1s.append(x1); xt0s.append(xt0); xt1s.append(xt1)

                # bias add + exp for all 16 score columns at once
                att_pre = spool.tile([P, 2 * B], F32, tag="att_pre")
                nc.vector.tensor_add(att_pre[:, :], at_ps[:, 0:2 * B], biasT)
                att = spool.tile([P, 2 * B], F32, tag="att")
                nc.scalar.activation(att[:, :], att_pre[:, :], mybir.ActivationFunctionType.Exp)

                # denominator broadcast over partitions: accumulate both s-tiles on PE
                nc.tensor.matmul(at_ps[:, 2 * B:3 * B], ones, att[:, 0:B], start=True, stop=False)
                nc.tensor.matmul(at_ps[:, 2 * B:3 * B], ones, att[:, B:2 * B], start=False, stop=True)
                rec = spool.tile([P, B], F32, tag="rec")
                nc.vector.reciprocal(rec[:, :], at_ps[:, 2 * B:3 * B])

                # xa[d,b] = sum_s X[s,d] a[s,b]  (accumulate the two s-tiles)
                for b in range(B):
                    nc.tensor.matmul(xo_ps[:, b:b + 1], x0s[b][:, :], att[:, b:b + 1], start=True, stop=False)
                    nc.tensor.matmul(xo_ps[:, b:b + 1], x1s[b][:, :], att[:, B + b:B + b + 1], start=False, stop=True)
                xa_sb = spool.tile([P, B], F32, tag="xa")
                nc.vector.tensor_mul(xa_sb[:, :], xo_ps[:, 0:B], rec[:, :])

                # out[e,b] = sum_d Wv[d,e] xa[d,b]; residual add; store
                nc.tensor.matmul(xo_ps[:, B:2 * B], wv[j], xa_sb[:, :], start=True, stop=True)
                fin = spool.tile([P, B], F32, tag="fin")
                nc.vector.tensor_add(fin[:, :], xo_ps[:, B:2 * B], cnt_j)
                nc.sync.dma_start(out=out_t[j, :, :], in_=fin[:, :])
    nc.finalize()
    return nc


def _get_nc():
    if "nc" not in _NC_CACHE:
        _NC_CACHE["nc"] = _build_nc()
    return _NC_CACHE["nc"]


def _pos_bias_f32():
    """t5_position_bucket exactly as the reference computes it (same jnp ops on the
    in-process default jax backend), sliced to the window."""
    if "pos" not in _NC_CACHE:
        import jax.numpy as jnp
        NUM_BUCKETS, MAX_DISTANCE = 32, 128
        n = (S - 1) - jnp.arange(S)
        max_exact = NUM_BUCKETS // 2
        is_small = n < max_exact
        large = max_exact + (
            jnp.log(jnp.maximum(n, 1).astype(jnp.float32) / max_exact)
            / np.log(MAX_DISTANCE / max_exact)
            * (NUM_BUCKETS - max_exact)
        ).astype(jnp.int32)
        large = jnp.minimum(large, NUM_BUCKETS - 1)
        pos = jnp.where(is_small, n, large).astype(jnp.float32)
        _NC_CACHE["pos"] = np.asarray(pos)[CUTOFF:]  # [W]
    return _NC_CACHE["pos"]


def kernel(**inputs) -> np.ndarray:
    t = int(np.asarray(inputs["t"]))
    assert t == T, f"kernel hardcoded for t={T}, got {t}"
    content_t = np.ascontiguousarray(np.asarray(inputs["content_t"], dtype=np.float32))
    time_mask = np.asarray(inputs["time_mask"])
    cache = np.asarray(inputs["cache"], dtype=np.float32)
    Wq = np.asarray(inputs["Wq"], dtype=np.float32)
    Wk = np.asarray(inputs["Wk"], dtype=np.float32)
    Wv = np.asarray(inputs["Wv"], dtype=np.float32)
    pos_param = np.float32(np.asarray(inputs["pos_param"]))

    # Per-position additive bias for the window: -pos_param*bucket only.
    # The reference's masked_fill sequence (1->0, then every 0->NEG) sets ALL
    # positions to NEG, a uniform shift softmax cancels -- time_mask is a no-op.
    del time_mask
    pos = _pos_bias_f32()                                   # [W]
    posb = (-pos_param * pos).astype(np.float32)            # [W]
    bias_col = posb.reshape(2, P).transpose(1, 0)           # [p, jt]
    bias_t = np.ascontiguousarray(
        np.broadcast_to(bias_col[:, :, None], (P, 2, B)).reshape(P, 2 * B)
    )  # [p, jt*8+b]

    win = cache[:, CUTOFF:T, :].reshape(B, W - 1, H, P)      # [B, 255, H, 128]
    cnt_h = content_t.reshape(B, H, P)                       # [B, H, 128]
    wkt_full = (Wk.transpose(0, 2, 1) / np.float32(np.sqrt(128.0))).astype(np.float32)

    ones = np.ones((P, P), np.float32)
    ident = np.eye(P, dtype=np.float32)

    in_maps = []
    for c in range(NCORES):
        h0 = HPC * c
        x_host = np.empty((B, HPC, W, P), np.float32)
        for j in range(HPC):
            x_host[:, j, : W - 1, :] = win[:, :, h0 + j, :]
            x_host[:, j, W - 1, :] = cnt_h[:, h0 + j, :]
        cnt_host = np.ascontiguousarray(
            cnt_h[:, h0:h0 + HPC, :].transpose(2, 1, 0).reshape(P, HPC * B)
        )  # [d, j*8+b]
        blocks = [ident, ones, bias_t, cnt_host]
        for j in range(HPC):
            blocks += [Wq[h0 + j], wkt_full[h0 + j], Wv[h0 + j]]
        consts_host = np.ascontiguousarray(np.concatenate(blocks, axis=1), dtype=np.float32)
        in_maps.append({"x": x_host, "consts": consts_host})

    nc = _get_nc()
    res = run_bass_kernel_spmd(nc, in_maps, list(range(NCORES)), **_RUN_KWARGS)
    _NC_CACHE["last_results"] = res
    outs = np.stack([np.asarray(res.results[c]["out"]) for c in range(NCORES)])
    # outs: [core, j, d, b] -> out_full[b, (2c+j)*128 + d]
    out_full = outs.transpose(3, 0, 1, 2).reshape(B, H * P)
    return out_full.astype(np.float32)


_RUN_KWARGS = {}  # test harness may set {"trace": True, "tmpdir": ...}



# revision 7
# speedup vs baseline: 2.5572x; 2.5572x over previous
"""Bass/Trainium2 kernel for nn_BiChannelAttention (single-query local-window attention).

Math (per batch b, head h, with S=2049, window W=256, cutoff=S-W=1793):
  Positions before the cutoff receive a -1e6 additive mask, so after softmax their
  weight is exactly 0.0 in fp32 (exp underflows). Only the last W positions matter.
  The reference's masked_fill sequence (1->0, then every 0->NEG) sets ALL positions
  to NEG -- a uniform shift softmax cancels, so time_mask is a no-op.
  bk shifts every score of a batch equally (q . bk) -- also cancelled by softmax.
  bv contributes exactly bv to the output (attn weights sum to 1) -> folded into
  the residual constant on the host.

Per (b, h): window rows X [W=256, 128] (last 255 cache rows + content row):
    q   = cnt_h @ Wq_h + bq                  (128)
    kq  = (Wk_h/sqrt(128))^T q               (128)   <- folds Wk into q
    sc  = X kq - pos_param * bucket(s)       (256)
    a   = exp(sc)      (no max-subtraction: unmasked scores are O(1))
    xa  = X^T a ;  den = sum(a)
    out = (Wv_h^T xa) / den + cnt_h + bv_h   (128)

Sharding: tensor-parallel over heads, 2 heads per core x 8 cores.

Engine plan per core (~110 instructions vs ~900 in the naive version):
  - X uploaded twice in fp8e4 (score layout X^T [d, (j,b,s)] and value layout
    [s128, (b,j,st,d)]), ~1MB/core total; weights/constants bf16/fp32.
  - scores: 32 matmuls, stationary = 128-col fp8 X^T tile (FWL), moving = kq [128,8]
    -> psum [s128, 8 batches x 8 cols]; the useful column of block b is col 9b.
  - exp+bias fused in 4 scalar.activation ops reading the strided psum columns.
  - denominator: ones-matmul broadcast; normalization deferred to the output.
  - value: 32 matmuls, stationary = fp8 X tile, moving = one attn column.
"""

import sys
import numpy as np
import ml_dtypes

for _p in ("/opt/trn_rl_repo", "/root/.axon_site/_ro/trn_rl_repo"):
    if _p not in sys.path:
        sys.path.insert(0, _p)

import concourse.bass as bass
import concourse.bacc as bacc
import concourse.mybir as mybir
from concourse.tile import TileContext
from concourse.bass_utils import run_bass_kernel_spmd

F32 = mybir.dt.float32
BF16 = mybir.dt.bfloat16
FP8 = mybir.dt.float8e4
NP_FP8 = ml_dtypes.float8_e4m3
NP_BF16 = ml_dtypes.bfloat16

P = 128          # partitions / head_dim
B = 8            # batch
H = 16           # heads total
HPC = 2          # heads per core
NCORES = 8
T = 2048
S = T + 1
W = 256          # local attention window
CUTOFF = S - W   # 1793
EXP = mybir.ActivationFunctionType.Exp
CPY = mybir.ActivationFunctionType.Copy

_NC_CACHE = {}


def _build_nc():
    nc = bacc.Bacc(None, target_bir_lowering=False, debug=False)
    xt_in = nc.declare_dram_parameter("xt", [HPC, P, B * W], FP8, isOutput=False)
    xn_in = nc.declare_dram_parameter("xn", [P, B * HPC * 2 * P], FP8, isOutput=False)
    wgt_in = nc.declare_dram_parameter("wgt", [P, 6 * P], BF16, isOutput=False)
    cnb_in = nc.declare_dram_parameter("cnb", [P, HPC * B], BF16, isOutput=False)
    cns_in = nc.declare_dram_parameter("cns", [P, 4 + HPC * B], F32, isOutput=False)
    out_t = nc.declare_dram_parameter("out", [P, HPC * B], F32, isOutput=True)

    with TileContext(nc) as tc:
        with (
            tc.tile_pool(name="xts", bufs=2) as xtpool,
            tc.tile_pool(name="xns", bufs=2) as xnpool,
            tc.tile_pool(name="small", bufs=2) as spool,
            tc.tile_pool(name="att", bufs=4) as apool,
            tc.tile_pool(name="ps_sc", bufs=2, space="PSUM") as pssc,
            tc.tile_pool(name="ps_sm", bufs=1, space="PSUM") as pssm,
            tc.tile_pool(name="ps_o", bufs=2, space="PSUM") as pso,
        ):
            # constants first (small, needed by the q/kq matmuls)
            cns = spool.tile([P, 4 + HPC * B], F32, tag="cns")
            nc.sync.dma_start(out=cns[:, :], in_=cns_in[:, :])
            cnb = spool.tile([P, HPC * B], BF16, tag="cnb")
            nc.sync.dma_start(out=cnb[:, :], in_=cnb_in[:, :])
            wgt = spool.tile([P, 6 * P], BF16, tag="wgt")
            nc.sync.dma_start(out=wgt[:, :], in_=wgt_in[:, :])

            xts, xns = [], []
            for j in range(HPC):
                xt_j = xtpool.tile([P, B * W], FP8, tag=f"xt{j}")
                nc.sync.dma_start(out=xt_j[:, :], in_=xt_in[j, :, :])
                xts.append(xt_j)
            for j in range(HPC):
                xn_j = xnpool.tile([P, B * 2 * P], FP8, tag=f"xn{j}")
                nc.scalar.dma_start(
                    out=xn_j[:, :], in_=xn_in[:, j * B * 2 * P:(j + 1) * B * 2 * P]
                )
                xns.append(xn_j)

            ones = spool.tile([P, P], BF16, tag="ones")
            nc.gpsimd.memset(ones[:, :], 1.0)

            fin = spool.tile([P, HPC * B], F32, tag="fin")

            for j in range(HPC):
                wq = wgt[:, (3 * j) * P:(3 * j + 1) * P]
                wkt = wgt[:, (3 * j + 1) * P:(3 * j + 2) * P]
                wv = wgt[:, (3 * j + 2) * P:(3 * j + 3) * P]

                # q[e,b] = sum_d Wq[d,e] cnt[d,b]  (+bq via activation bias)
                qk_ps = pssm.tile([P, 2 * B], F32, tag="qk")
                nc.tensor.matmul(qk_ps[:, 0:B], wq, cnb[:, j * B:(j + 1) * B],
                                 start=True, stop=True)
                q_sb = spool.tile([P, B], BF16, tag="q")
                nc.vector.tensor_scalar_add(q_sb[:, :], qk_ps[:, 0:B],
                                            cns[:, j:j + 1])
                # kq[d,b] = sum_e (Wk[d,e]/sqrt(hd)) q[e,b]
                nc.tensor.matmul(qk_ps[:, B:2 * B], wkt, q_sb[:, :],
                                 start=True, stop=True)
                kq_sb = spool.tile([P, B], BF16, tag="kq")
                nc.vector.tensor_copy(kq_sb[:, :], qk_ps[:, B:2 * B])

                # scores: per (st, b): [s128, 8] block; useful col of block b is 9b
                a_sb = []
                for st in range(2):
                    sc_ps = pssc.tile([P, B * B], F32, tag="sc")
                    for b in range(B):
                        nc.tensor.matmul(
                            sc_ps[:, b * B:(b + 1) * B],
                            xts[j][:, b * W + st * P: b * W + st * P + P],
                            kq_sb[:, :], start=True, stop=True)
                    a = apool.tile([P, B], BF16, tag=f"a{j}{st}")
                    nc.scalar.activation(a[:, :], sc_ps[:, 0:B * B:B + 1], EXP,
                                         bias=cns[:, 2 + st:3 + st])
                    a_sb.append(a)

                # denominator broadcast over partitions (accumulate both s-tiles)
                dn_ps = pssm.tile([P, B], F32, tag="dn")
                nc.tensor.matmul(dn_ps[:, :], ones, a_sb[0][:, :],
                                 start=True, stop=False)
                nc.tensor.matmul(dn_ps[:, :], ones, a_sb[1][:, :],
                                 start=False, stop=True)
                rec = spool.tile([P, B], F32, tag="rec")
                nc.vector.reciprocal(rec[:, :], dn_ps[:, :])

                # xa[d,b] = sum_s X[s,d] a[s,b]
                xa_ps = pso.tile([P, 2 * B], F32, tag="xa")
                for b in range(B):
                    c0 = b * 2 * P
                    nc.tensor.matmul(xa_ps[:, b:b + 1], xns[j][:, c0:c0 + P],
                                     a_sb[0][:, b:b + 1], start=True, stop=False)
                    nc.tensor.matmul(xa_ps[:, b:b + 1], xns[j][:, c0 + P:c0 + 2 * P],
                                     a_sb[1][:, b:b + 1], start=False, stop=True)
                xa_sb = spool.tile([P, B], BF16, tag="xa_sb")
                nc.vector.tensor_copy(xa_sb[:, :], xa_ps[:, 0:B])

                # out[e,b] = (Wv[d,e] xa[d,b]) / den + (cnt + bv)
                nc.tensor.matmul(xa_ps[:, B:2 * B], wv, xa_sb[:, :],
                                 start=True, stop=True)
                t_sb = spool.tile([P, B], F32, tag="t")
                nc.vector.tensor_mul(t_sb[:, :], xa_ps[:, B:2 * B], rec[:, :])
                nc.vector.tensor_add(fin[:, j * B:(j + 1) * B], t_sb[:, :],
                                     cns[:, 4 + j * B:4 + (j + 1) * B])

            nc.sync.dma_start(out=out_t[:, :], in_=fin[:, :])
    nc.finalize()
    return nc


def _get_nc():
    if "nc" not in _NC_CACHE:
        _NC_CACHE["nc"] = _build_nc()
    return _NC_CACHE["nc"]


def _pos_window_f32():
    """t5_position_bucket(S) with the reference's ops in numpy, sliced to window."""
    if "pos" not in _NC_CACHE:
        NUM_BUCKETS, MAX_DISTANCE = 32, 128
        n = (S - 1) - np.arange(S)
        max_exact = NUM_BUCKETS // 2
        is_small = n < max_exact
        large = max_exact + (
            np.log(np.maximum(n, 1).astype(np.float32) / max_exact)
            / np.log(MAX_DISTANCE / max_exact)
            * (NUM_BUCKETS - max_exact)
        ).astype(np.int32)
        large = np.minimum(large, NUM_BUCKETS - 1)
        pos = np.where(is_small, n, large).astype(np.float32)
        _NC_CACHE["pos"] = pos[CUTOFF:]  # [W]
    return _NC_CACHE["pos"]


def kernel(**inputs) -> np.ndarray:
    t = int(np.asarray(inputs["t"]))
    assert t == T, f"kernel hardcoded for t={T}, got {t}"
    content_t = np.asarray(inputs["content_t"], dtype=np.float32)
    cache = np.asarray(inputs["cache"], dtype=np.float32)
    Wq = np.asarray(inputs["Wq"], dtype=np.float32)
    bq = np.asarray(inputs["bq"], dtype=np.float32)
    Wk = np.asarray(inputs["Wk"], dtype=np.float32)
    Wv = np.asarray(inputs["Wv"], dtype=np.float32)
    bv = np.asarray(inputs["bv"], dtype=np.float32)
    pos_param = np.float32(np.asarray(inputs["pos_param"]))
    # time_mask: the reference's masked_fill chain biases every position equally
    # (softmax-invariant); bk shifts all of a batch's scores equally. Both no-ops.

    posb = (-pos_param * _pos_window_f32()).astype(np.float32)      # [W]

    # window rows per (b, s, h, d), s=0..254 from cache, s=255 = content row
    win = np.empty((B, W, H, P), np.float32)
    win[:, :W - 1] = cache[:, CUTOFF:T, :].reshape(B, W - 1, H, P)
    win[:, W - 1] = content_t.reshape(B, H, P)
    win8 = win.astype(NP_FP8)

    wkt_full = (Wk.transpose(0, 2, 1) / np.float32(np.sqrt(128.0))).astype(np.float32)
    cnt_h = content_t.reshape(B, H, P)

    in_maps = []
    for c in range(NCORES):
        h0 = HPC * c
        wc = win8[:, :, h0:h0 + HPC, :]                              # [B, W, 2, P]
        # xt[j, d, b*W+s] = wc[b, s, j, d]
        xt_host = np.ascontiguousarray(
            wc.transpose(2, 3, 0, 1).reshape(HPC, P, B * W))
        # xn[s128, ((j*B+b)*2+st)*P+d] = wc[b, st*128+s128, j, d]
        xn_host = np.ascontiguousarray(
            wc.reshape(B, 2, P, HPC, P).transpose(2, 3, 0, 1, 4)
            .reshape(P, B * HPC * 2 * P))
        wgt_host = np.empty((P, 6 * P), np.float32)
        for j in range(HPC):
            wgt_host[:, (3 * j) * P:(3 * j + 1) * P] = Wq[h0 + j]
            wgt_host[:, (3 * j + 1) * P:(3 * j + 2) * P] = wkt_full[h0 + j]
            wgt_host[:, (3 * j + 2) * P:(3 * j + 3) * P] = Wv[h0 + j]
        # cnb[d, j*B+b] = cnt[b, h0+j, d]
        cnb_host = np.ascontiguousarray(
            cnt_h[:, h0:h0 + HPC, :].transpose(2, 1, 0).reshape(P, HPC * B))
        cns_host = np.empty((P, 4 + HPC * B), np.float32)
        for j in range(HPC):
            cns_host[:, j] = bq[h0 + j]
        cns_host[:, 2] = posb[0:P]
        cns_host[:, 3] = posb[P:W]
        for j in range(HPC):
            cns_host[:, 4 + j * B:4 + (j + 1) * B] = (
                cnt_h[:, h0 + j, :] + bv[h0 + j][None, :]).T
        in_maps.append({
            "xt": xt_host,
            "xn": xn_host.astype(NP_FP8),
            "wgt": wgt_host.astype(NP_BF16),
            "cnb": cnb_host.astype(NP_BF16),
            "cns": cns_host,
        })

    nc = _get_nc()
    res = run_bass_kernel_spmd(nc, in_maps, list(range(NCORES)), **_RUN_KWARGS)
    _NC_CACHE["last_results"] = res
    # out[e, j*B+b] per core -> out_full[b, (2c+j)*128+e]
    out_full = np.empty((B, H * P), np.float32)
    for c in range(NCORES):
        oc = np.asarray(res.results[c]["out"])
        for j in range(HPC):
            out_full[:, (HPC * c + j) * P:(HPC * c + j + 1) * P] = \
                oc[:, j * B:(j + 1) * B].T
    return out_full


_RUN_KWARGS = {}  # test harness may set {"trace": True, "tmpdir": ...}


# revision 10
# speedup vs baseline: 2.8636x; 1.1198x over previous
"""Bass/Trainium2 kernel for nn_BiChannelAttention (single-query local-window attention).

Math (per batch b, head h, with S=2049, window W=256, cutoff=S-W=1793):
  Positions before the cutoff receive a -1e6 additive mask, so after softmax their
  weight is exactly 0.0 in fp32 (exp underflows). Only the last W positions matter.
  The reference's masked_fill sequence (1->0, then every 0->NEG) sets ALL positions
  to NEG -- a uniform shift softmax cancels, so time_mask is a no-op.
  bk shifts every score of a batch equally (q . bk) -- cancelled by softmax.
  bv contributes exactly bv to the output (attn weights sum to 1) -> folded into
  the residual constant on the host.

Per (b, h): window rows X [W=256, 128] (last 255 cache rows + content row):
    q   = cnt_h @ Wq_h + bq                  (128)
    kq  = (Wk_h/sqrt(128))^T q               (128)   <- folds Wk into q
    sc  = X kq - pos_param * bucket(s)       (256)
    a   = exp(sc)      (no max-subtraction: unmasked scores are O(1))
    xa  = X^T a ;  den = sum(a)
    out = (Wv_h^T xa) / den + cnt_h + bv_h   (128)

Sharding: tensor-parallel over heads, 2 heads per core x 8 cores.

Engine/latency plan per core:
  - 6 input DMAs fired in parallel at t=0 across 4 queues (sync/scalar/vector/gpsimd)
    so the serialized ~1-2us HBM fixed costs overlap.
  - X uploaded twice in fp8e4 (score layout X^T [j, d, (b,s)] and value layout
    [s128, (j,b,st,d)]), ~1MB/core; weights bf16, residual/bias consts fp32.
  - PE warm-up: dummy matmuls during the DMA window lift the HAM clock gate
    (1.2 -> 2.4 GHz) before the real matmuls arrive.
  - q/kq for both heads in one K-stacked accumulation chain (zero-padded rhs).
  - scores: 32 matmuls, stationary = 128-col fp8 X^T tile, moving = kq [128,8]
    -> psum [s128, 8 batches x 8 cols]; the useful column of block b is col 9b.
  - exp+bias fused in 4 scalar.activation ops reading the strided psum columns.
  - denominator: ones-matmul broadcast; normalization deferred to the output.
  - value: 32 matmuls, stationary = fp8 X tile, moving = one attn column.
"""

import sys
import numpy as np
import ml_dtypes

for _p in ("/opt/trn_rl_repo", "/root/.axon_site/_ro/trn_rl_repo"):
    if _p not in sys.path:
        sys.path.insert(0, _p)

import concourse.bass as bass
import concourse.bacc as bacc
import concourse.mybir as mybir
from concourse.tile import TileContext
from concourse.bass_utils import run_bass_kernel_spmd

F32 = mybir.dt.float32
BF16 = mybir.dt.bfloat16
FP8 = mybir.dt.float8e4
NP_FP8 = ml_dtypes.float8_e4m3
NP_BF16 = ml_dtypes.bfloat16

P = 128          # partitions / head_dim
B = 8            # batch
H = 16           # heads total
HPC = 2          # heads per core
NCORES = 8
T = 2048
S = T + 1
W = 256          # local attention window
CUTOFF = S - W   # 1793
EXP = mybir.ActivationFunctionType.Exp
N_WARM = 28      # dummy matmuls to lift the HAM clock gate (~3.4us at 1.2GHz)

_NC_CACHE = {}


def _build_nc():
    nc = bacc.Bacc(None, target_bir_lowering=False, debug=False)
    xt_in = nc.declare_dram_parameter("xt", [HPC, P, B * W], FP8, isOutput=False)
    xn_in = nc.declare_dram_parameter("xn", [P, HPC * B * 2 * P], FP8, isOutput=False)
    # wgtc: [Wq,WkT,Wv] x 2 heads | zero-padded cnt blocks for the K-stacked q
    wgt_cols = 6 * P + 2 * HPC * B
    wgt_in = nc.declare_dram_parameter("wgt", [P, wgt_cols], BF16, isOutput=False)
    # cns: bq2 [0:16] | exp bias per s-tile [16:18] | cnt+bv residual [18:34]
    cns_in = nc.declare_dram_parameter("cns", [P, 2 * HPC * B + 2], F32, isOutput=False)
    out_t = nc.declare_dram_parameter("out", [HPC, P, B], F32, isOutput=True)

    JB = HPC * B

    with TileContext(nc) as tc:
        with (
            tc.tile_pool(name="xts", bufs=2) as xtpool,
            tc.tile_pool(name="xns", bufs=2) as xnpool,
            tc.tile_pool(name="small", bufs=2) as spool,
            tc.tile_pool(name="att", bufs=4) as apool,
            tc.tile_pool(name="ps_sc", bufs=2, space="PSUM") as pssc,
            tc.tile_pool(name="ps_sm", bufs=1, space="PSUM") as pssm,
            tc.tile_pool(name="ps_o", bufs=2, space="PSUM") as pso,
            tc.tile_pool(name="ps_w", bufs=1, space="PSUM") as psw,
        ):
            # ---- input DMAs, all issued at t=0 on parallel queues ----
            cns = spool.tile([P, 2 * JB + 2], F32, tag="cns")
            nc.sync.dma_start(out=cns[:, :], in_=cns_in[:, :])
            wgt = spool.tile([P, wgt_cols], BF16, tag="wgt")
            nc.sync.dma_start(out=wgt[:, :], in_=wgt_in[:, :])

            xts = []
            for j, eng in zip(range(HPC), (nc.scalar, nc.sync)):
                xt_j = xtpool.tile([P, B * W], FP8, tag=f"xt{j}")
                eng.dma_start(out=xt_j[:, :], in_=xt_in[j, :, :])
                xts.append(xt_j)
            xns = []
            for j in range(HPC):
                xn_j = xnpool.tile([P, B * 2 * P], FP8, tag=f"xn{j}")
                nc.gpsimd.dma_start(
                    out=xn_j[:, :], in_=xn_in[:, j * B * 2 * P:(j + 1) * B * 2 * P]
                )
                xns.append(xn_j)

            # ---- PE warm-up on an engine-local constant ----
            ones = spool.tile([P, P], BF16, tag="ones")
            nc.vector.memset(ones[:, :], 1.0)
            junk_ps = psw.tile([P, P], F32, tag="junk")
            for _ in range(N_WARM):
                nc.tensor.matmul(junk_ps[:, :], ones, ones, start=True, stop=True)

            # ---- q/kq for both heads: K-stacked accumulation ----
            wq = [wgt[:, (3 * j) * P:(3 * j + 1) * P] for j in range(HPC)]
            wkt = [wgt[:, (3 * j + 1) * P:(3 * j + 2) * P] for j in range(HPC)]
            wv = [wgt[:, (3 * j + 2) * P:(3 * j + 3) * P] for j in range(HPC)]
            cnx = [wgt[:, 6 * P + j * JB:6 * P + (j + 1) * JB] for j in range(HPC)]

            qk_ps = pssm.tile([P, 2 * JB], F32, tag="qk")
            nc.tensor.matmul(qk_ps[:, 0:JB], wq[0], cnx[0], start=True, stop=False)
            nc.tensor.matmul(qk_ps[:, 0:JB], wq[1], cnx[1], start=False, stop=True)
            q_sb = spool.tile([P, JB], BF16, tag="q")
            nc.vector.tensor_add(q_sb[:, :], qk_ps[:, 0:JB], cns[:, 0:JB])
            for j in range(HPC):
                nc.tensor.matmul(qk_ps[:, JB + j * B:JB + (j + 1) * B], wkt[j],
                                 q_sb[:, j * B:(j + 1) * B], start=True, stop=True)
            kq_sb = spool.tile([P, JB], BF16, tag="kq")
            nc.vector.tensor_copy(kq_sb[:, :], qk_ps[:, JB:2 * JB])

            for j in range(HPC):
                # scores: per (st, b): [s128, 8] block; useful col of block b is 9b
                a_sb = []
                for st in range(2):
                    sc_ps = pssc.tile([P, B * B], F32, tag="sc")
                    for b in range(B):
                        nc.tensor.matmul(
                            sc_ps[:, b * B:(b + 1) * B],
                            xts[j][:, b * W + st * P: b * W + st * P + P],
                            kq_sb[:, j * B:(j + 1) * B], start=True, stop=True)
                    a = apool.tile([P, B], BF16, tag=f"a{j}{st}")
                    nc.scalar.activation(a[:, :], sc_ps[:, 0:B * B:B + 1], EXP,
                                         bias=cns[:, JB + st:JB + st + 1])
                    a_sb.append(a)

                # denominator broadcast over partitions (accumulate both s-tiles)
                dn_ps = pssm.tile([P, B], F32, tag="dn")
                nc.tensor.matmul(dn_ps[:, :], ones, a_sb[0][:, :],
                                 start=True, stop=False)
                nc.tensor.matmul(dn_ps[:, :], ones, a_sb[1][:, :],
                                 start=False, stop=True)
                rec = spool.tile([P, B], F32, tag="rec")
                nc.vector.reciprocal(rec[:, :], dn_ps[:, :])

                # xa[d,b] = sum_s X[s,d] a[s,b]
                xa_ps = pso.tile([P, 2 * B], F32, tag="xa")
                for b in range(B):
                    c0 = b * 2 * P
                    nc.tensor.matmul(xa_ps[:, b:b + 1], xns[j][:, c0:c0 + P],
                                     a_sb[0][:, b:b + 1], start=True, stop=False)
                    nc.tensor.matmul(xa_ps[:, b:b + 1], xns[j][:, c0 + P:c0 + 2 * P],
                                     a_sb[1][:, b:b + 1], start=False, stop=True)
                xa_sb = spool.tile([P, B], BF16, tag="xa_sb")
                nc.vector.tensor_copy(xa_sb[:, :], xa_ps[:, 0:B])

                # out[e,b] = (Wv[d,e] xa[d,b]) / den + (cnt + bv)
                nc.tensor.matmul(xa_ps[:, B:2 * B], wv[j], xa_sb[:, :],
                                 start=True, stop=True)
                t_sb = spool.tile([P, B], F32, tag="t")
                nc.vector.tensor_mul(t_sb[:, :], xa_ps[:, B:2 * B], rec[:, :])
                fin = spool.tile([P, B], F32, tag=f"fin{j}")
                nc.vector.tensor_add(fin[:, :], t_sb[:, :],
                                     cns[:, JB + 2 + j * B:JB + 2 + (j + 1) * B])
                nc.sync.dma_start(out=out_t[j, :, :], in_=fin[:, :])
    nc.finalize()
    return nc


def _get_nc():
    if "nc" not in _NC_CACHE:
        _NC_CACHE["nc"] = _build_nc()
    return _NC_CACHE["nc"]


def _pos_window_f32():
    """t5_position_bucket(S) with the reference's ops in numpy, sliced to window."""
    if "pos" not in _NC_CACHE:
        NUM_BUCKETS, MAX_DISTANCE = 32, 128
        n = (S - 1) - np.arange(S)
        max_exact = NUM_BUCKETS // 2
        is_small = n < max_exact
        large = max_exact + (
            np.log(np.maximum(n, 1).astype(np.float32) / max_exact)
            / np.log(MAX_DISTANCE / max_exact)
            * (NUM_BUCKETS - max_exact)
        ).astype(np.int32)
        large = np.minimum(large, NUM_BUCKETS - 1)
        pos = np.where(is_small, n, large).astype(np.float32)
        _NC_CACHE["pos"] = pos[CUTOFF:]  # [W]
    return _NC_CACHE["pos"]


def kernel(**inputs) -> np.ndarray:
    t = int(np.asarray(inputs["t"]))
    assert t == T, f"kernel hardcoded for t={T}, got {t}"
    content_t = np.asarray(inputs["content_t"], dtype=np.float32)
    cache = np.asarray(inputs["cache"], dtype=np.float32)
    Wq = np.asarray(inputs["Wq"], dtype=np.float32)
    bq = np.asarray(inputs["bq"], dtype=np.float32)
    Wk = np.asarray(inputs["Wk"], dtype=np.float32)
    Wv = np.asarray(inputs["Wv"], dtype=np.float32)
    bv = np.asarray(inputs["bv"], dtype=np.float32)
    pos_param = np.float32(np.asarray(inputs["pos_param"]))
    # time_mask: the reference's masked_fill chain biases every position equally
    # (softmax-invariant); bk shifts all of a batch's scores equally. Both no-ops.

    posb = (-pos_param * _pos_window_f32()).astype(np.float32)      # [W]

    # window rows per (b, s, h, d), s=0..254 from cache, s=255 = content row
    win = np.empty((B, W, H, P), np.float32)
    win[:, :W - 1] = cache[:, CUTOFF:T, :].reshape(B, W - 1, H, P)
    win[:, W - 1] = content_t.reshape(B, H, P)
    win8 = win.astype(NP_FP8)

    wkt_full = (Wk.transpose(0, 2, 1) / np.float32(np.sqrt(128.0))).astype(np.float32)
    cnt_h = content_t.reshape(B, H, P)
    JB = HPC * B
    wgt_cols = 6 * P + 2 * JB

    in_maps = []
    for c in range(NCORES):
        h0 = HPC * c
        wc = win8[:, :, h0:h0 + HPC, :]                              # [B, W, 2, P]
        # xt[j, d, b*W+s] = wc[b, s, j, d]
        xt_host = np.ascontiguousarray(
            wc.transpose(2, 3, 0, 1).reshape(HPC, P, B * W))
        # xn[s128, ((j*B+b)*2+st)*P+d] = wc[b, st*128+s128, j, d]
        xn_host = np.ascontiguousarray(
            wc.reshape(B, 2, P, HPC, P).transpose(2, 3, 0, 1, 4)
            .reshape(P, HPC * B * 2 * P))
        wgt_host = np.zeros((P, wgt_cols), np.float32)
        for j in range(HPC):
            wgt_host[:, (3 * j) * P:(3 * j + 1) * P] = Wq[h0 + j]
            wgt_host[:, (3 * j + 1) * P:(3 * j + 2) * P] = wkt_full[h0 + j]
            wgt_host[:, (3 * j + 2) * P:(3 * j + 3) * P] = Wv[h0 + j]
            # zero-padded cnt block for the K-stacked q: block j holds cnt_j in
            # its own (j,b) columns, zeros elsewhere
            wgt_host[:, 6 * P + j * JB + j * B:6 * P + j * JB + (j + 1) * B] = \
                cnt_h[:, h0 + j, :].T
        cns_host = np.empty((P, 2 * JB + 2), np.float32)
        for j in range(HPC):
            cns_host[:, j * B:(j + 1) * B] = bq[h0 + j][:, None]
            cns_host[:, JB + 2 + j * B:JB + 2 + (j + 1) * B] = (
                cnt_h[:, h0 + j, :] + bv[h0 + j][None, :]).T
        cns_host[:, JB] = posb[0:P]
        cns_host[:, JB + 1] = posb[P:W]
        in_maps.append({
            "xt": xt_host,
            "xn": xn_host,
            "wgt": wgt_host.astype(NP_BF16),
            "cns": cns_host,
        })

    nc = _get_nc()
    res = run_bass_kernel_spmd(nc, in_maps, list(range(NCORES)), **_RUN_KWARGS)
    _NC_CACHE["last_results"] = res
    # out[j, e, b] per core -> out_full[b, (2c+j)*128+e]
    out_full = np.empty((B, H * P), np.float32)
    for c in range(NCORES):
        oc = np.asarray(res.results[c]["out"])
        for j in range(HPC):
            out_full[:, (HPC * c + j) * P:(HPC * c + j + 1) * P] = oc[j].T
    return out_full


_RUN_KWARGS = {}  # test harness may set {"trace": True, "tmpdir": ...}
